# revision 1
# baseline (speedup 1.0000x reference)
# Trainium2 Bass kernel for nn_Democracy_loss (supervised-contrastive loss).
#
# Strategy: the dominant cost is the first embed GEMM
#   h_pre = X @ W1,  X: [320, 120000] f32, W1: [120000, 128] f32
# (215 MB of input read; everything downstream is tiny). We shard the
# CONTRACTION dim K=120000 across the 8 cores (15000 rows each) so W1 is
# *not* replicated: every input byte is read exactly once (~27 MB/core).
# Each core computes a partial h_pre^T = W1_c^T @ X_c^T into PSUM
# ([128, 320] f32, fits one bank) and returns it. The host sums the 8
# partials, applies b1/relu, the tiny 320x128x128 second GEMM, and the
# data-dependent ragged pos/neg loss grouping (integer metadata, host-side).
#
# Device layout: per core one packed DRAM input [128, 118, 448] where
# packed[p, t, 0:320]  = X^T[k0 + t*128 + p, :]   (moving operand tile)
# packed[p, t, 320:448] = W1[k0 + t*128 + p, :]   (stationary operand tile)
# so each chunk of k-tiles is ONE contiguous-per-partition dma_start
# (~2.3 MB), and each k-tile is one matmul:
#   psum[n, m] += lhsT(W1-tile [128k,128n]).T @ rhs(XT-tile [128k,320m])
# K per core = 15000, zero-padded to 118*128 = 15104.

import sys

import numpy as np

for _p in ("/opt/trn_rl_repo",):
    if _p not in sys.path:
        sys.path.append(_p)

NF, NC_SAMPLES, B_TOTAL = 256, 64, 320
IN_DIM = 120000
HID = 128
N_CORES = 8
K_PER_CORE = IN_DIM // N_CORES          # 15000
KTILES = (K_PER_CORE + 127) // 128      # 118 (padded to 15104)
K_PAD = KTILES * 128
NK_CHUNK = 10
PACK_W = B_TOTAL + HID                  # 448

TEMPERATURE = 0.07
BASE_TEMPERATURE = 1.0
EPS = 1e-12

_BUILT = None          # cached compiled Bass program
LAST_EXEC_NS = None    # set when tracing is enabled (see run_device)


def _build_bass():
    """Build + compile the per-core Bass program (same program on all cores)."""
    global _BUILT
    if _BUILT is not None:
        return _BUILT
    import concourse.bacc as bacc
    import concourse.bass as bass
    import concourse.mybir as mybir
    import concourse.tile as tile

    f32 = mybir.dt.float32
    nc = bacc.Bacc(
        "TRN2", target_bir_lowering=False, debug=False, num_devices=N_CORES
    )
    xw = nc.dram_tensor("xw", [128, KTILES, PACK_W], f32, kind="ExternalInput")
    out = nc.dram_tensor("out", [128, B_TOTAL], f32, kind="ExternalOutput")

    with tile.TileContext(nc) as tc:
        with (
            tc.tile_pool(name="io", bufs=4) as io_pool,
            tc.tile_pool(name="res", bufs=1) as res_pool,
            tc.tile_pool(name="acc", bufs=1, space=bass.MemorySpace.PSUM) as pp,
        ):
            psum = pp.tile([128, B_TOTAL], f32)
            t = 0
            while t < KTILES:
                nk = min(NK_CHUNK, KTILES - t)
                chunk = io_pool.tile([128, NK_CHUNK, PACK_W], f32, tag="chunk")
                nc.sync.dma_start(chunk[:, :nk, :], xw[:, t : t + nk, :])
                for j in range(nk):
                    nc.tensor.matmul(
                        psum[:, :],
                        chunk[:, j, B_TOTAL:PACK_W],   # lhsT: W1 k-tile [128, 128]
                        chunk[:, j, 0:B_TOTAL],        # rhs: X^T k-tile [128, 320]
                        start=(t + j == 0),
                        stop=(t + j == KTILES - 1),
                    )
                t += nk
            out_sb = res_pool.tile([128, B_TOTAL], f32)
            nc.vector.tensor_copy(out_sb[:, :], psum[:, :])
            nc.sync.dma_start(out[:, :], out_sb[:, :])

    nc.compile()
    _BUILT = nc
    return nc


def _pack_inputs(X, W1):
    """X: [320, 120000] f32, W1: [120000, 128] f32 -> 8 per-core packed maps."""
    XT = np.ascontiguousarray(X.T)  # [120000, 320]
    in_maps = []
    for c in range(N_CORES):
        ks = c * K_PER_CORE
        ke = ks + K_PER_CORE
        buf = np.zeros((K_PAD, PACK_W), np.float32)
        buf[:K_PER_CORE, :B_TOTAL] = XT[ks:ke]
        buf[:K_PER_CORE, B_TOTAL:] = W1[ks:ke]
        packed = np.ascontiguousarray(
            buf.reshape(KTILES, 128, PACK_W).transpose(1, 0, 2)
        )
        in_maps.append({"xw": packed})
    return in_maps


def run_device(X, W1, trace=False):
    """Run the sharded partial-GEMM on the 8 NeuronCores; return h_pre [320,128] f64."""
    global LAST_EXEC_NS
    from concourse.bass_utils import run_bass_kernel_spmd

    nc = _build_bass()
    in_maps = _pack_inputs(X, W1)
    res = run_bass_kernel_spmd(nc, in_maps, list(range(N_CORES)), trace=trace)
    if res.exec_time_ns is not None:
        LAST_EXEC_NS = res.exec_time_ns
    acc = np.zeros((128, B_TOTAL), np.float64)
    for c in range(N_CORES):
        acc += res.results[c]["out"].astype(np.float64)
    return acc.T  # [320, 128] pre-activation (no bias yet)


def _anchor_loss(anchor_e, pos_e, neg_e):
    # mirrors the reference exactly (computed in float64 on host)
    T = TEMPERATURE
    posn = pos_e / np.maximum(
        np.sqrt(np.sum(pos_e * pos_e, axis=-2, keepdims=True)), EPS
    )
    negn = neg_e / np.maximum(
        np.sqrt(np.sum(neg_e * neg_e, axis=-2, keepdims=True)), EPS
    )
    an = anchor_e / np.maximum(np.sqrt(np.sum(anchor_e * anchor_e)), EPS)
    A = (negn @ an) / T
    m = np.max(A)
    log_sum = np.log(np.sum(np.exp(A - m)))
    num = (posn @ an) / T
    return -(T / BASE_TEMPERATURE) * np.mean(num - log_sum)


def _host_loss(E, lab, cf, iff, cc, ic):
    Ef, Ec = E[:NF], E[NF:]
    lc = lab[ic]
    lf = lab[iff]
    wrong_idx = np.nonzero((cc[:, 0] != lc) & (cc[:, 1] == lc))[0]
    corr_idx = np.nonzero(cc[:, 0] == lc)[0]
    corrf_idx = np.nonzero(cf[:, 0] == lf)[0]
    uniq = np.unique(np.concatenate([cc[wrong_idx].ravel(), cc[corr_idx].ravel()]))
    pos_of = {int(c): corrf_idx[cf[corrf_idx, 0] == c] for c in uniq}
    losses = []
    for i in wrong_idx:
        top1, top2 = int(cc[i, 0]), int(cc[i, 1])
        neg_extra = wrong_idx[cc[wrong_idx, 0] == top2]
        neg_e = np.concatenate([Ef[pos_of[top1]], Ec[neg_extra]], axis=0)
        pos_e = Ef[pos_of[top2]]
        if pos_e.shape[0] == 0 or neg_e.shape[0] == 0:
            continue
        losses.append(_anchor_loss(Ec[i], pos_e, neg_e))
    for i in corr_idx:
        pos_e = Ef[pos_of[int(cc[i, 0])]]
        neg_e = Ef[pos_of[int(cc[i, 1])]]
        if pos_e.shape[0] == 0 or neg_e.shape[0] == 0:
            continue
        losses.append(_anchor_loss(Ec[i], pos_e, neg_e))
    if losses:
        return np.mean(np.stack(losses))
    return np.float32(0.0)


def kernel(
    label,
    samples_of_further_pairs,
    class_of_further_pair,
    idx_further_pair,
    samples_of_closest_pairs,
    class_of_closest_pair,
    idx_closest_pair,
    W1,
    b1,
    W2,
    b2,
):
    import os

    X = np.concatenate(
        [
            np.asarray(samples_of_further_pairs, np.float32).reshape(NF, -1),
            np.asarray(samples_of_closest_pairs, np.float32).reshape(NC_SAMPLES, -1),
        ],
        axis=0,
    )  # [320, 120000]
    W1 = np.ascontiguousarray(np.asarray(W1, np.float32))

    h_pre = run_device(X, W1, trace=bool(os.environ.get("KERNEL_TRACE")))
    h = np.maximum(h_pre + np.asarray(b1, np.float64), 0.0)
    E = h @ np.asarray(W2, np.float64) + np.asarray(b2, np.float64)  # [320, 128]

    loss = _host_loss(
        E,
        np.asarray(label).astype(np.int64),
        np.asarray(class_of_further_pair).astype(np.int64),
        np.asarray(idx_further_pair).astype(np.int64),
        np.asarray(class_of_closest_pair).astype(np.int64),
        np.asarray(idx_closest_pair).astype(np.int64),
    )
    return np.asarray(loss, dtype=np.float32)
